# revision 2
# baseline (speedup 1.0000x reference)
"""Self-contained Trainium2 Bass kernel: 16-head attention with RoPE (B=2, S=2048, D=2048).

Sharding: 8 cores = 2 (batch) x 4 (head groups of 4 heads / 512 cols).
Per core: QKV projections for its head group -> RoPE -> causal attention ->
AllGather of attention outputs (X^T) within the 4-core batch group ->
column-sharded output projection. Host assembles by concatenation only.

Pipeline per q-chunk c (512 wide):
  hT(c+1) prefetch, K(c)+RoPE, V(s-tiles 4c..4c+3), Q(c)+RoPE,
  attention(c) [causal k-tiles 0..4c+3], AllGather(c), outproj(c-1)

Dataflow is fully "transposed" so no on-chip transposes are needed:
  hiddenT [d, s] (host-pretransposed, bf16), one merged DMA per chunk
  QT/KT   [dh, s] per head  (projection emits head-dim-major directly)
  S^T     [k, q] scores; causal mask added INSIDE the PSUM accumulation
          group via a tiny eye^T @ tri matmul (no vector op before exp)
  P^T     [k, q] = exp(S^T) bf16; diagonal tiles computed on [c0:] only
  rowsums: in-place partial-width bf16 pairwise tree over P^T tiles,
           then partition-reduce+broadcast via an all-ones matmul
  O^T     [dh, q] = V^T @ P^T  (partial-width on diagonal tiles)
  X^T     AllGather on first axis; outproj reloads peer blocks with one
          merged DMA and reuses the local ot tiles directly.
RoPE de-interleave is folded into a host-side row permutation of Wq/Wk;
the rotation is 3 full-partition vector ops using a sign-folded sin table.
1/sqrt(DH) is folded into the Q rope tables.
"""

import math
from contextlib import ExitStack

import numpy as np
import ml_dtypes

B, S, D, H, DH = 2, 2048, 2048, 16, 128
NCORES = 8
GPC = 4            # cores per tensor-parallel group
HPC = H // GPC     # heads per core (4)
CW = HPC * DH      # 512 columns per core
NEG = -1e9
BF = ml_dtypes.bfloat16
QCH = 512          # q-chunk (moving free dim)
NQC = S // QCH     # 4
NDT = D // 128     # 16 d-tiles
NST = S // 128     # 16 s-tiles

REPLICA_GROUPS = [[0, 1, 2, 3], [4, 5, 6, 7]]

_built = {}


def _build(causal: bool, use_bias: bool):
    import concourse.bass as bass
    import concourse.tile as tile
    from concourse import bacc, mybir
    from concourse.tile_rust import add_dep_helper

    f32, bf16 = mybir.dt.float32, mybir.dt.bfloat16
    EXP = mybir.ActivationFunctionType.Exp
    IDN = mybir.ActivationFunctionType.Identity

    nc = bacc.Bacc("TRN2", target_bir_lowering=False, debug=False,
                   num_devices=NCORES)

    hT_d = nc.dram_tensor("hiddenT", [D, S], bf16, kind="ExternalInput")
    wq_d = nc.dram_tensor("wqT", [D, CW], bf16, kind="ExternalInput")
    wk_d = nc.dram_tensor("wkT", [D, CW], bf16, kind="ExternalInput")
    wv_d = nc.dram_tensor("wvT", [D, CW], bf16, kind="ExternalInput")
    wo_d = nc.dram_tensor("woT", [D, CW], bf16, kind="ExternalInput")
    cq_d = nc.dram_tensor("cq", [128, S], bf16, kind="ExternalInput")
    sq_d = nc.dram_tensor("sq", [128, S], bf16, kind="ExternalInput")
    ck_d = nc.dram_tensor("ck", [128, S], bf16, kind="ExternalInput")
    sk_d = nc.dram_tensor("sk", [128, S], bf16, kind="ExternalInput")
    ey_d = nc.dram_tensor("eye128", [128, 128], bf16, kind="ExternalInput")
    if use_bias:
        bq_d = nc.dram_tensor("bqp", [128, HPC], f32, kind="ExternalInput")
        bk_d = nc.dram_tensor("bkp", [128, HPC], f32, kind="ExternalInput")
        bv_d = nc.dram_tensor("bv2", [1, CW], f32, kind="ExternalInput")
        bo_d = nc.dram_tensor("bo2", [1, CW], f32, kind="ExternalInput")
    if causal:
        dm_d = nc.dram_tensor("dmask", [128, 128], bf16, kind="ExternalInput")
    else:
        mT_d = nc.dram_tensor("maskT", [S, S], bf16, kind="ExternalInput")
    out_d = nc.dram_tensor("out", [S, CW], f32, kind="ExternalOutput")

    def blk(ap_2d, g):
        """3D [128, g, n] view of a [(g*128), n] DRAM AP."""
        return ap_2d.rearrange("(g p) n -> p g n", p=128)

    with tile.TileContext(nc) as tc, ExitStack() as ctx:
        hp = ctx.enter_context(tc.tile_pool(name="hp", bufs=2))
        xp = ctx.enter_context(tc.tile_pool(name="xp", bufs=1))
        wp = ctx.enter_context(tc.tile_pool(name="wp", bufs=4))
        qkp = ctx.enter_context(tc.tile_pool(name="qkp", bufs=2 * HPC))
        vp = ctx.enter_context(tc.tile_pool(name="vp", bufs=NST))
        cst = ctx.enter_context(tc.tile_pool(name="cst", bufs=1))
        ptp = ctx.enter_context(tc.tile_pool(name="ptp", bufs=3))
        rp = ctx.enter_context(tc.tile_pool(name="rp", bufs=2))
        op = ctx.enter_context(tc.tile_pool(name="op", bufs=3))
        ps_a = ctx.enter_context(tc.tile_pool(name="ps_a", bufs=3, space="PSUM"))
        ps_s = ctx.enter_context(tc.tile_pool(name="ps_s", bufs=3, space="PSUM"))
        ps_pv = ctx.enter_context(tc.tile_pool(name="ps_pv", bufs=2, space="PSUM"))
        dram = ctx.enter_context(tc.tile_pool(name="dram", bufs=1, space="DRAM"))

        # ---- first-needed weights first: wk + hT(chunk0) interleaved in
        # sub-DMAs (smallest first) so the first K-proj matmuls start early.
        ones_sb = cst.tile([128, 128], bf16, tag="ones", name="ones_sb")
        nc.vector.memset(ones_sb[:], 1.0)
        wk_sb = wp.tile([128, NDT * CW], bf16, tag="w", name="wk_sb")
        wv_sb = wp.tile([128, NDT * CW], bf16, tag="w", name="wv_sb")
        wq_sb = wp.tile([128, NDT * CW], bf16, tag="w", name="wq_sb")
        wo_sb = wp.tile([128, NDT * CW], bf16, tag="w", name="wo_sb")
        hTs = {0: hp.tile([128, NDT * QCH], bf16, tag="hT", name="hT0")}
        for g0, g1 in ((0, 2), (2, 6), (6, 11), (11, 16)):
            gsl = slice(g0 * 128, g1 * 128)
            nc.sync.dma_start(
                wk_sb[:].rearrange("p (g n) -> p g n", g=NDT)[:, g0:g1, :],
                blk(wk_d[gsl, :], g1 - g0))
            nc.sync.dma_start(
                hTs[0][:].rearrange("p (g n) -> p g n", g=NDT)[:, g0:g1, :],
                blk(hT_d[gsl, 0:QCH], g1 - g0))
        ck_sb = cst.tile([128, S], bf16, tag="ck", name="ck_sb")
        sk_sb = cst.tile([128, S], bf16, tag="sk", name="sk_sb")
        nc.sync.dma_start(ck_sb[:], ck_d[:])
        nc.sync.dma_start(sk_sb[:], sk_d[:])
        nc.sync.dma_start(
            wq_sb[:].rearrange("p (g n) -> p g n", g=NDT), blk(wq_d[:, :], NDT))
        cq_sb = cst.tile([128, S], bf16, tag="cq", name="cq_sb")
        sq_sb = cst.tile([128, S], bf16, tag="sq", name="sq_sb")
        nc.sync.dma_start(cq_sb[:], cq_d[:])
        nc.sync.dma_start(sq_sb[:], sq_d[:])
        eye_sb = cst.tile([128, 128], bf16, tag="eye", name="eye_sb")
        nc.sync.dma_start(eye_sb[:], ey_d[:])
        if causal:
            tri_sb = cst.tile([128, 128], bf16, tag="tri", name="tri_sb")
            nc.sync.dma_start(tri_sb[:], dm_d[:])
        nc.sync.dma_start(
            wv_sb[:].rearrange("p (g n) -> p g n", g=NDT), blk(wv_d[:, :], NDT))
        if use_bias:
            bq_sb = cst.tile([128, HPC], f32, tag="bq", name="bq_sb")
            bk_sb = cst.tile([128, HPC], f32, tag="bk", name="bk_sb")
            bv_sb = cst.tile([1, CW], f32, tag="bv", name="bv_sb")
            bo_sb = cst.tile([1, CW], f32, tag="bo", name="bo_sb")
            nc.sync.dma_start(bq_sb[:], bq_d[:])
            nc.sync.dma_start(bk_sb[:], bk_d[:])
            nc.sync.dma_start(bv_sb[:], bv_d[:])
            nc.sync.dma_start(bo_sb[:], bo_d[:])
            bvb_sb = cst.tile([128, CW], f32, tag="bvb", name="bvb_sb")
            bob_sb = cst.tile([128, CW], f32, tag="bob", name="bob_sb")
            nc.gpsimd.partition_broadcast(bvb_sb[:], bv_sb[0:1, :])
            nc.gpsimd.partition_broadcast(bob_sb[:], bo_sb[0:1, :])
        # Wo streams in behind everything else
        nc.sync.dma_start(
            wo_sb[:].rearrange("p (g n) -> p g n", g=NDT), blk(wo_d[:, :], NDT))

        def w_at(w, dt, lo, sz):
            return w[:, dt * CW + lo: dt * CW + lo + sz]

        # persistent KT (written chunk by chunk; all history needed) and V;
        # QT is per-chunk only
        ktr = [qkp.tile([128, S], bf16, tag="qk", name=f"ktr{m}", bufs=HPC)
               for m in range(HPC)]
        v_sb = [None] * NST

        def proj_chunk_qk(w_sb, b_sb, c_sb, s_sb, dsts, dsls, hTc, c, prefix,
                          after_vec=None):
            """Project chunk c of Q or K into dsts[m][:, dsls[m]] + RoPE.

            RoPE: rows 0:64 hold a ("real"), 64:128 hold b ("imag").
              new_a = a*cos - b*sin ; new_b = b*cos + a*sin
            With t1 = [b; a] (half-swapped copy) and s2 = [-sin; +sin]:
              dst = dst * c  +  t1 * s2      (3 vector ops, full partitions)
            """
            csl = slice(c * QCH, (c + 1) * QCH)  # rope-table columns
            for m in range(HPC):
                dst, dsl = dsts[m], dsls[m]
                ps = ps_a.tile([128, QCH], f32, tag="mm",
                               name=f"{prefix}ps{m}_{c}")
                for dt in range(NDT):
                    nc.tensor.matmul(ps[:], w_at(w_sb, dt, m * 128, 128),
                                     hTc[:, dt * QCH:(dt + 1) * QCH],
                                     start=(dt == 0), stop=(dt == NDT - 1))
                if use_bias:
                    nc.scalar.activation(dst[:, dsl], ps[:], IDN,
                                         bias=b_sb[:, m:m + 1])
                else:
                    nc.scalar.activation(dst[:, dsl], ps[:], IDN)
                t1 = rp.tile([128, QCH], bf16, tag="t1", name=f"{prefix}t1{m}_{c}",
                             bufs=3)
                nc.gpsimd.dma_start(t1[0:64, :], dst[64:128, dsl])
                nc.gpsimd.dma_start(t1[64:128, :], dst[0:64, dsl])
                v1 = nc.vector.tensor_mul(t1[:], t1[:], s_sb[:, csl])
                if after_vec is not None and m == 0:
                    # keep this chunk's RoPE vector ops behind the previous
                    # chunk's attention vector ops in the DVE queue
                    add_dep_helper(v1.ins, after_vec.ins, sync=False,
                                   reason="rope after prev attn vec")
                nc.vector.tensor_mul(dst[:, dsl], dst[:, dsl], c_sb[:, csl])
                nc.vector.tensor_add(dst[:, dsl], dst[:, dsl], t1[:])

        def proj_chunk_v(hTc, c):
            last = None
            for sti in range(4):
                st = 4 * c + sti
                ps = ps_a.tile([128, CW], f32, tag="mm", name=f"psv{st}")
                for dt in range(NDT):
                    last = nc.tensor.matmul(
                        ps[:], hTc[:, dt * QCH + sti * 128: dt * QCH + (sti + 1) * 128],
                        w_at(wv_sb, dt, 0, CW),
                        start=(dt == 0), stop=(dt == NDT - 1))
                vt = vp.tile([128, CW], bf16, tag="v", name=f"v{st}")
                if use_bias:
                    nc.vector.tensor_add(vt[:], ps[:], bvb_sb[:])
                else:
                    nc.scalar.activation(vt[:], ps[:], IDN)
                v_sb[st] = vt
            return last

        def attention_chunk(qc, qtrc, qoff, W, allow_pop=True,
                            start_anchor=None):
            """Attention for q-window [qc*QCH+qoff, +W); one AllGather piece."""
            qbase = qc * QCH + qoff
            agin = dram.tile([CW, W], bf16, tag=f"agin{qc}_{qoff}",
                             name=f"agin{qc}_{qoff}")
            agout = dram.tile([D, W], bf16, tag=f"agout{qc}_{qoff}",
                              name=f"agout{qc}_{qoff}")
            last_mm = start_anchor
            last_vec = None
            nk = (qbase + W) // 128 if causal else NST
            for h in range(HPC):
                # backfill PE bubbles (rope latency at h==0, exp pacing
                # otherwise) with pending outproj blocks; never during the
                # last chunk -- it delays the tail AllGather
                if allow_pop:
                    for _ in range(len(oblocks) // (HPC - h)):
                        oblocks.pop(0)(last_mm)
                pv = ps_pv.tile([128, W], f32, tag="pv", name=f"pv{h}_{qc}_{qoff}")
                pts = []
                for ki in range(nk):
                    rel = 128 * ki - qbase if causal else -128
                    c0 = max(0, rel)
                    ss = ps_s.tile([128, W], f32, tag="s",
                                   name=f"ss{h}_{qc}_{qoff}_{ki}")
                    nc.tensor.matmul(
                        ss[:, c0:], ktr[h][:, ki * 128:(ki + 1) * 128],
                        qtrc[h][:, qoff + c0:qoff + W], start=True,
                        stop=not (causal and rel >= 0), skip_group_check=True)
                    if causal and rel >= 0:
                        # additive causal mask folded into the accumulation
                        # group: ss[:, c0:c0+128] += tri  (eye^T @ tri = tri)
                        nc.tensor.matmul(ss[:, c0:c0 + 128], eye_sb[:],
                                         tri_sb[:], start=False, stop=True,
                                         skip_group_check=True)
                    if not causal:
                        mt = ptp.tile([128, W], bf16, tag="mt",
                                      name=f"mt{h}_{qc}_{ki}", bufs=2)
                        nc.sync.dma_start(
                            mt[:], mT_d[ki * 128:(ki + 1) * 128,
                                        qbase:qbase + W])
                        nc.vector.tensor_add(ss[:], ss[:], mt[:])
                    pt = ptp.tile([128, W], bf16, tag="pt",
                                  name=f"pt{h}_{qc}_{qoff}_{ki}",
                                  bufs=12 if causal else 8)
                    nc.scalar.activation(pt[:, c0:], ss[:, c0:], EXP)
                    last_mm = nc.tensor.matmul(
                        pv[:, c0:], v_sb[ki][:, h * 128:(h + 1) * 128],
                        pt[:, c0:], start=(ki == 0), stop=(ki == nk - 1),
                        skip_group_check=True)
                    pts.append((pt, c0))
                # in-place partial-width pairwise tree; the last tile is kept
                # out of the eager tree so only ONE add remains after the
                # final exp (short critical path into the rowsum matmul).
                tail = pts[-1]
                pts = pts[:-1]
                while len(pts) > 1:
                    nxt = []
                    for i in range(0, len(pts) - 1, 2):
                        (a, ca), (b, cb) = pts[i], pts[i + 1]
                        nc.vector.tensor_add(a[:, cb:], a[:, cb:], b[:, cb:])
                        nxt.append((a, ca))
                    if len(pts) % 2:
                        nxt.append(pts[-1])
                    pts = nxt
                root = pts[0][0] if pts else tail[0]
                if pts and tail is not None:
                    nc.vector.tensor_add(root[:, tail[1]:], root[:, tail[1]:],
                                         tail[0][:, tail[1]:])
                # partition-reduce+broadcast the rowsums in one bf16 matmul
                sm = ps_s.tile([128, W], f32, tag="s", name=f"sm{h}_{qc}_{qoff}")
                nc.tensor.matmul(sm[:], ones_sb[:], root[:],
                                 start=True, stop=True)
                recb = op.tile([128, W], f32, tag="recb",
                               name=f"recb{h}_{qc}_{qoff}", bufs=2)
                nc.vector.reciprocal_approx_fast(out=recb[:], in_=sm[:])
                ot = op.tile([128, W], bf16, tag="ot", name=f"ot{h}_{qc}_{qoff}",
                             bufs=HPC + 2)
                last_vec = nc.vector.tensor_mul(ot[:], pv[:], recb[:])
                nc.sync.dma_start(agin[h * 128:(h + 1) * 128, :], ot[:])

            nc.gpsimd.collective_compute(
                "AllGather", mybir.AluOpType.bypass,
                replica_groups=REPLICA_GROUPS,
                ins=[agin[:].opt()], outs=[agout[:].opt()])
            return (agout, qoff, W), last_mm, last_vec

        oblocks = []          # pending outproj st4-block emitters (FIFO)

        def push_outproj(qc, piece):
            """Emit the xt reload DMAs for one AllGather piece and queue its
            outproj blocks for interleaving into later attention heads."""
            agout, o, w = piece
            xts = []
            for half in range(2):
                xt = xp.tile([128, 8 * w], bf16, tag="xt",
                             name=f"xt{qc}_{o}_{half}",
                             bufs=3 if causal else 2)
                nc.sync.dma_start(
                    xt[:].rearrange("p (g n) -> p g n", g=8),
                    blk(agout[half * 8 * 128:(half + 1) * 8 * 128, :], 8))
                xts.append(xt)

            def make_block(st4):
                def emit(after_mm):
                    ps = ps_a.tile([128, CW], f32, tag="mm",
                                   name=f"pso{qc}_{o}_{st4}")
                    last = None
                    for dt in range(NDT):
                        mm = last = nc.tensor.matmul(
                            ps[:],
                            xts[dt // 8][:, (dt % 8) * w + st4 * 128:
                                         (dt % 8) * w + (st4 + 1) * 128],
                            w_at(wo_sb, dt, 0, CW),
                            start=(dt == 0), stop=(dt == NDT - 1))
                        if dt == 0 and after_mm is not None:
                            # keep this block behind the attention matmuls
                            # it is meant to backfill (the static scheduler
                            # underestimates AllGather latency and would
                            # hoist it otherwise)
                            add_dep_helper(mm.ins, after_mm.ins, sync=True,
                                           reason="outproj backfill order")
                    row = qc * QCH + o + st4 * 128
                    of = op.tile([128, CW], f32, tag="of",
                                 name=f"of{qc}_{o}_{st4}", bufs=2)
                    if use_bias:
                        nc.vector.tensor_add(of[:], ps[:], bob_sb[:])
                    else:
                        nc.scalar.activation(of[:], ps[:], IDN)
                    nc.sync.dma_start(out_d[row:row + 128, :], of[:])
                    return last
                return emit

            for st4 in range(w // 128):
                oblocks.append(make_block(st4))

        # ---- main pipeline over q-chunks ----
        pieces, last_mms = {}, {}
        qtc_bufs = HPC + 1 if causal else 4 * HPC

        def proj_block(c, after_vec):
            if c + 1 < NQC:
                nsl = slice((c + 1) * QCH, (c + 2) * QCH)
                nxt = hp.tile([128, NDT * QCH], bf16, tag="hT",
                              name=f"hT{c + 1}")
                nc.sync.dma_start(
                    nxt[:].rearrange("p (g n) -> p g n", g=NDT),
                    blk(hT_d[:, nsl], NDT))
                hTs[c + 1] = nxt
            hTc = hTs[c][:]
            # K first: its RoPE chain (vector) overlaps the V+Q matmuls, so
            # the vector queue is clear when attention needs it.
            proj_chunk_qk(wk_sb, bk_sb if use_bias else None, ck_sb, sk_sb,
                          ktr, [slice(c * QCH, (c + 1) * QCH)] * HPC,
                          hTc, c, "k", after_vec)
            qtrc = [qkp.tile([128, QCH], bf16, tag="qtc", name=f"qtc{c}_{m}",
                             bufs=qtc_bufs) for m in range(HPC)]
            proj_chunk_qk(wq_sb, bq_sb if use_bias else None, cq_sb, sq_sb,
                          qtrc, [slice(0, QCH)] * HPC, hTc, c, "q")
            # V last: its matmuls cover the K/Q RoPE chains before attention
            vlast = proj_chunk_v(hTc, c)
            return qtrc, vlast

        HW = QCH // 2
        if causal:
            av = None
            for c in range(NQC):
                qtrc, vlast = proj_block(c, av)
                if c < NQC - 1:
                    pc, lm, av = attention_chunk(c, qtrc, 0, QCH,
                                                 start_anchor=vlast)
                    push_outproj(c, pc)
                else:
                    # split the last chunk so its first AllGather starts a
                    # half-chunk earlier and the tail shrinks
                    pa, _, _ = attention_chunk(c, qtrc, 0, HW, False)
                    push_outproj(c, pa)
                    pb, lm, av = attention_chunk(c, qtrc, HW, HW, False)
                    push_outproj(c, pb)
            prev = lm
            for emit in oblocks:
                prev = emit(prev)
            oblocks.clear()
        else:
            # non-causal: attention(c) needs the FULL K/V, so project
            # everything first, then run the attention/AG/outproj pipeline
            pv_list = [proj_block(c, None) for c in range(NQC)]
            for c in range(NQC):
                qtrc, vlast = pv_list[c]
                if c < NQC - 1:
                    pc, lm, _ = attention_chunk(c, qtrc, 0, QCH,
                                                start_anchor=vlast)
                    push_outproj(c, pc)
                else:
                    pa, _, _ = attention_chunk(c, qtrc, 0, HW, False)
                    push_outproj(c, pa)
                    pb, lm, _ = attention_chunk(c, qtrc, HW, HW, False)
                    push_outproj(c, pb)
            prev = lm
            for emit in oblocks:
                prev = emit(prev)
            oblocks.clear()

    nc.compile()
    return nc


def _get_built(causal: bool, use_bias: bool):
    key = (causal, use_bias)
    if key not in _built:
        _built[key] = _build(causal, use_bias)
    return _built[key]


def _prep_inputs(inputs, causal, use_bias):
    hs = np.asarray(inputs["hidden_states"], np.float32)
    fc = np.asarray(inputs["freqs_cis"], np.float32)
    Wq = np.asarray(inputs["Wq"], np.float32)
    Wk = np.asarray(inputs["Wk"], np.float32)
    Wv = np.asarray(inputs["Wv"], np.float32)
    Wo = np.asarray(inputs["Wo"], np.float32)
    bq = np.asarray(inputs["bq"], np.float32)
    bk = np.asarray(inputs["bk"], np.float32)
    bv = np.asarray(inputs["bv"], np.float32)
    bo = np.asarray(inputs["bo"], np.float32)

    # de-interleave permutation per 128-row head block: [0,2,..,126, 1,3,..,127]
    perm1 = np.concatenate([np.arange(0, DH, 2), np.arange(1, DH, 2)])
    permC = (np.arange(CW) // DH) * DH  # head base offsets
    perm = permC + perm1[np.arange(CW) % DH]

    scale = 1.0 / math.sqrt(DH)
    cos = np.concatenate([fc[:, :, 0].T, fc[:, :, 0].T])  # [128, S], dup halves
    sinh = fc[:, :, 1].T                                  # [64, S]
    sin2 = np.concatenate([-sinh, sinh])                  # [128, S], sign-folded
    cq = np.ascontiguousarray(cos * scale).astype(BF)
    sq = np.ascontiguousarray(sin2 * scale).astype(BF)
    ck = np.ascontiguousarray(cos).astype(BF)
    sk = np.ascontiguousarray(sin2).astype(BF)
    eye = np.eye(128, dtype=np.float32).astype(BF)

    if causal:
        tri = np.where(np.arange(128)[:, None] > np.arange(128)[None, :],
                       np.float32(NEG), np.float32(0.0)).astype(BF)
    else:
        maskT = np.ascontiguousarray(
            np.asarray(inputs["mask"], np.float32)[0, 0].T).astype(BF)

    hTb = [np.ascontiguousarray(hs[b].T).astype(BF) for b in range(B)]

    in_maps = []
    for c in range(NCORES):
        b, hg = divmod(c, GPC)
        sl = slice(CW * hg, CW * (hg + 1))
        wq_s = Wq[sl][perm]
        wk_s = Wk[sl][perm]
        m = {
            "hiddenT": hTb[b],
            "wqT": np.ascontiguousarray(wq_s.T).astype(BF),
            "wkT": np.ascontiguousarray(wk_s.T).astype(BF),
            "wvT": np.ascontiguousarray(Wv[sl].T).astype(BF),
            "woT": np.ascontiguousarray(Wo[sl].T).astype(BF),
            "cq": cq, "sq": sq, "ck": ck, "sk": sk, "eye128": eye,
        }
        if use_bias:
            m["bqp"] = np.ascontiguousarray(
                bq[sl][perm].reshape(HPC, 128).T).astype(np.float32)
            m["bkp"] = np.ascontiguousarray(
                bk[sl][perm].reshape(HPC, 128).T).astype(np.float32)
            m["bv2"] = bv[sl].reshape(1, CW).astype(np.float32)
            m["bo2"] = bo[sl].reshape(1, CW).astype(np.float32)
        if causal:
            m["dmask"] = tri
        else:
            m["maskT"] = maskT
        in_maps.append(m)
    return in_maps


def _is_causal(mask):
    mask = np.asarray(mask, np.float32)
    if mask.shape != (1, 1, S, S):
        return False
    m = mask[0, 0]
    expect = np.triu(np.full((S, S), np.float32(NEG)), k=1)
    return bool(np.array_equal(m, expect))


def run_on_cores(inputs, trace=False):
    """Compile+run; returns BassKernelResults."""
    from concourse.bass_utils import run_bass_kernel_spmd
    causal = _is_causal(inputs["mask"])
    use_bias = any(
        np.any(np.asarray(inputs[k])) for k in ("bq", "bk", "bv", "bo"))
    in_maps = _prep_inputs(inputs, causal, use_bias)
    nc = _get_built(causal, use_bias)
    r = run_bass_kernel_spmd(nc, in_maps, list(range(NCORES)), trace=trace)
    return r


def kernel(**inputs) -> np.ndarray:
    r = run_on_cores(inputs)
    out = np.empty((B, S, D), np.float32)
    for c in range(NCORES):
        b, hg = divmod(c, GPC)
        out[b, :, CW * hg:CW * (hg + 1)] = r.results[c]["out"]
    return out


# revision 3
# speedup vs baseline: 1.0181x; 1.0181x over previous
"""Self-contained Trainium2 Bass kernel: 16-head attention with RoPE (B=2, S=2048, D=2048).

Sharding: 8 cores = 2 (batch) x 4 (head groups of 4 heads / 512 cols).
Per core: QKV projections for its head group -> RoPE -> causal attention ->
AllGather of attention outputs (X^T) within the 4-core batch group ->
column-sharded output projection. Host assembles by concatenation only.

Pipeline per q-chunk c (512 wide):
  hT(c+1) prefetch, K(c)+RoPE, V(s-tiles 4c..4c+3), Q(c)+RoPE,
  attention(c) [causal k-tiles 0..4c+3], AllGather(c), outproj(c-1)

Dataflow is fully "transposed" so no on-chip transposes are needed:
  hiddenT [d, s] (host-pretransposed, bf16), one merged DMA per chunk
  QT/KT   [dh, s] per head  (projection emits head-dim-major directly)
  S^T     [k, q] scores; causal mask added INSIDE the PSUM accumulation
          group via a tiny eye^T @ tri matmul (no vector op before exp)
  P^T     [k, q] = exp(S^T) bf16; diagonal tiles computed on [c0:] only
  rowsums: in-place partial-width bf16 pairwise tree over P^T tiles,
           then partition-reduce+broadcast via an all-ones matmul
  O^T     [dh, q] = V^T @ P^T  (partial-width on diagonal tiles)
  X^T     AllGather on first axis; outproj reloads peer blocks with one
          merged DMA and reuses the local ot tiles directly.
RoPE de-interleave is folded into a host-side row permutation of Wq/Wk;
the rotation is 3 full-partition vector ops using a sign-folded sin table.
1/sqrt(DH) is folded into the Q rope tables.
"""

import math
from contextlib import ExitStack

import numpy as np
import ml_dtypes

B, S, D, H, DH = 2, 2048, 2048, 16, 128
NCORES = 8
GPC = 4            # cores per tensor-parallel group
HPC = H // GPC     # heads per core (4)
CW = HPC * DH      # 512 columns per core
NEG = -1e9
BF = ml_dtypes.bfloat16
QCH = 512          # q-chunk (moving free dim)
NQC = S // QCH     # 4
NDT = D // 128     # 16 d-tiles
NST = S // 128     # 16 s-tiles

REPLICA_GROUPS = [[0, 1, 2, 3], [4, 5, 6, 7]]

_built = {}


def _build(causal: bool, use_bias: bool):
    import concourse.bass as bass
    import concourse.tile as tile
    from concourse import bacc, mybir
    from concourse.tile_rust import add_dep_helper

    f32, bf16 = mybir.dt.float32, mybir.dt.bfloat16
    EXP = mybir.ActivationFunctionType.Exp
    IDN = mybir.ActivationFunctionType.Identity

    nc = bacc.Bacc("TRN2", target_bir_lowering=False, debug=False,
                   num_devices=NCORES)

    hT_d = nc.dram_tensor("hiddenT", [D, S], bf16, kind="ExternalInput")
    wq_d = nc.dram_tensor("wqT", [D, CW], bf16, kind="ExternalInput")
    wk_d = nc.dram_tensor("wkT", [D, CW], bf16, kind="ExternalInput")
    wv_d = nc.dram_tensor("wvT", [D, CW], bf16, kind="ExternalInput")
    wo_d = nc.dram_tensor("woT", [D, CW], bf16, kind="ExternalInput")
    cq_d = nc.dram_tensor("cq", [128, S], bf16, kind="ExternalInput")
    sq_d = nc.dram_tensor("sq", [128, S], bf16, kind="ExternalInput")
    ck_d = nc.dram_tensor("ck", [128, S], bf16, kind="ExternalInput")
    sk_d = nc.dram_tensor("sk", [128, S], bf16, kind="ExternalInput")
    ey_d = nc.dram_tensor("eye128", [128, 128], bf16, kind="ExternalInput")
    if use_bias:
        bq_d = nc.dram_tensor("bqp", [128, HPC], f32, kind="ExternalInput")
        bk_d = nc.dram_tensor("bkp", [128, HPC], f32, kind="ExternalInput")
        bv_d = nc.dram_tensor("bv2", [1, CW], f32, kind="ExternalInput")
        bo_d = nc.dram_tensor("bo2", [1, CW], f32, kind="ExternalInput")
    if causal:
        dm_d = nc.dram_tensor("dmask", [128, 128], bf16, kind="ExternalInput")
    else:
        mT_d = nc.dram_tensor("maskT", [S, S], bf16, kind="ExternalInput")
    out_d = nc.dram_tensor("out", [S, CW], f32, kind="ExternalOutput")

    def blk(ap_2d, g):
        """3D [128, g, n] view of a [(g*128), n] DRAM AP."""
        return ap_2d.rearrange("(g p) n -> p g n", p=128)

    with tile.TileContext(nc) as tc, ExitStack() as ctx:
        hp = ctx.enter_context(tc.tile_pool(name="hp", bufs=2))
        xp = ctx.enter_context(tc.tile_pool(name="xp", bufs=1))
        wp = ctx.enter_context(tc.tile_pool(name="wp", bufs=4))
        qkp = ctx.enter_context(tc.tile_pool(name="qkp", bufs=2 * HPC))
        vp = ctx.enter_context(tc.tile_pool(name="vp", bufs=NST))
        cst = ctx.enter_context(tc.tile_pool(name="cst", bufs=1))
        ptp = ctx.enter_context(tc.tile_pool(name="ptp", bufs=3))
        rp = ctx.enter_context(tc.tile_pool(name="rp", bufs=2))
        op = ctx.enter_context(tc.tile_pool(name="op", bufs=3))
        ps_a = ctx.enter_context(tc.tile_pool(name="ps_a", bufs=3, space="PSUM"))
        ps_s = ctx.enter_context(tc.tile_pool(name="ps_s", bufs=3, space="PSUM"))
        ps_pv = ctx.enter_context(tc.tile_pool(name="ps_pv", bufs=2, space="PSUM"))
        dram = ctx.enter_context(tc.tile_pool(name="dram", bufs=1, space="DRAM"))

        # ---- first-needed weights first: wk + hT(chunk0) interleaved in
        # sub-DMAs (smallest first) so the first K-proj matmuls start early.
        ones_sb = cst.tile([128, 128], bf16, tag="ones", name="ones_sb")
        nc.vector.memset(ones_sb[:], 1.0)
        wk_sb = wp.tile([128, NDT * CW], bf16, tag="w", name="wk_sb")
        wv_sb = wp.tile([128, NDT * CW], bf16, tag="w", name="wv_sb")
        wq_sb = wp.tile([128, NDT * CW], bf16, tag="w", name="wq_sb")
        wo_sb = wp.tile([128, NDT * CW], bf16, tag="w", name="wo_sb")
        hTs = {0: hp.tile([128, NDT * QCH], bf16, tag="hT", name="hT0")}
        for g0, g1 in ((0, 1), (1, 3), (3, 7), (7, 11), (11, 16)):
            gsl = slice(g0 * 128, g1 * 128)
            nc.sync.dma_start(
                wk_sb[:].rearrange("p (g n) -> p g n", g=NDT)[:, g0:g1, :],
                blk(wk_d[gsl, :], g1 - g0))
            # hT0 rides the scalar HWDGE queue, in parallel with wk on sync
            nc.scalar.dma_start(
                hTs[0][:].rearrange("p (g n) -> p g n", g=NDT)[:, g0:g1, :],
                blk(hT_d[gsl, 0:QCH], g1 - g0))
        ck_sb = cst.tile([128, S], bf16, tag="ck", name="ck_sb")
        sk_sb = cst.tile([128, S], bf16, tag="sk", name="sk_sb")
        nc.sync.dma_start(ck_sb[:], ck_d[:])
        nc.sync.dma_start(sk_sb[:], sk_d[:])
        nc.sync.dma_start(
            wq_sb[:].rearrange("p (g n) -> p g n", g=NDT), blk(wq_d[:, :], NDT))
        cq_sb = cst.tile([128, S], bf16, tag="cq", name="cq_sb")
        sq_sb = cst.tile([128, S], bf16, tag="sq", name="sq_sb")
        nc.sync.dma_start(cq_sb[:], cq_d[:])
        nc.sync.dma_start(sq_sb[:], sq_d[:])
        eye_sb = cst.tile([128, 128], bf16, tag="eye", name="eye_sb")
        nc.sync.dma_start(eye_sb[:], ey_d[:])
        if causal:
            tri_sb = cst.tile([128, 128], bf16, tag="tri", name="tri_sb")
            nc.sync.dma_start(tri_sb[:], dm_d[:])
        nc.sync.dma_start(
            wv_sb[:].rearrange("p (g n) -> p g n", g=NDT), blk(wv_d[:, :], NDT))
        if use_bias:
            bq_sb = cst.tile([128, HPC], f32, tag="bq", name="bq_sb")
            bk_sb = cst.tile([128, HPC], f32, tag="bk", name="bk_sb")
            bv_sb = cst.tile([1, CW], f32, tag="bv", name="bv_sb")
            bo_sb = cst.tile([1, CW], f32, tag="bo", name="bo_sb")
            nc.sync.dma_start(bq_sb[:], bq_d[:])
            nc.sync.dma_start(bk_sb[:], bk_d[:])
            nc.sync.dma_start(bv_sb[:], bv_d[:])
            nc.sync.dma_start(bo_sb[:], bo_d[:])
            bvb_sb = cst.tile([128, CW], f32, tag="bvb", name="bvb_sb")
            bob_sb = cst.tile([128, CW], f32, tag="bob", name="bob_sb")
            nc.gpsimd.partition_broadcast(bvb_sb[:], bv_sb[0:1, :])
            nc.gpsimd.partition_broadcast(bob_sb[:], bo_sb[0:1, :])
        # Wo streams in behind everything else
        nc.sync.dma_start(
            wo_sb[:].rearrange("p (g n) -> p g n", g=NDT), blk(wo_d[:, :], NDT))

        def w_at(w, dt, lo, sz):
            return w[:, dt * CW + lo: dt * CW + lo + sz]

        # persistent KT (written chunk by chunk; all history needed) and V;
        # QT is per-chunk only
        ktr = [qkp.tile([128, S], bf16, tag="qk", name=f"ktr{m}", bufs=HPC)
               for m in range(HPC)]
        v_sb = [None] * NST

        def proj_chunk_qk(w_sb, b_sb, c_sb, s_sb, dsts, dsls, hTc, c, prefix,
                          after_vec=None):
            """Project chunk c of Q or K into dsts[m][:, dsls[m]] + RoPE.

            RoPE: rows 0:64 hold a ("real"), 64:128 hold b ("imag").
              new_a = a*cos - b*sin ; new_b = b*cos + a*sin
            With t1 = [b; a] (half-swapped copy) and s2 = [-sin; +sin]:
              dst = dst * c  +  t1 * s2      (3 vector ops, full partitions)
            """
            csl = slice(c * QCH, (c + 1) * QCH)  # rope-table columns
            for m in range(HPC):
                dst, dsl = dsts[m], dsls[m]
                ps = ps_a.tile([128, QCH], f32, tag="mm",
                               name=f"{prefix}ps{m}_{c}")
                for dt in range(NDT):
                    nc.tensor.matmul(ps[:], w_at(w_sb, dt, m * 128, 128),
                                     hTc[:, dt * QCH:(dt + 1) * QCH],
                                     start=(dt == 0), stop=(dt == NDT - 1))
                if use_bias:
                    nc.scalar.activation(dst[:, dsl], ps[:], IDN,
                                         bias=b_sb[:, m:m + 1])
                else:
                    nc.scalar.activation(dst[:, dsl], ps[:], IDN)
                t1 = rp.tile([128, QCH], bf16, tag="t1", name=f"{prefix}t1{m}_{c}",
                             bufs=3)
                nc.gpsimd.dma_start(t1[0:64, :], dst[64:128, dsl])
                nc.gpsimd.dma_start(t1[64:128, :], dst[0:64, dsl])
                v1 = nc.vector.tensor_mul(t1[:], t1[:], s_sb[:, csl])
                if after_vec is not None and m == 0:
                    # keep this chunk's RoPE vector ops behind the previous
                    # chunk's attention vector ops in the DVE queue
                    add_dep_helper(v1.ins, after_vec.ins, sync=False,
                                   reason="rope after prev attn vec")
                nc.vector.tensor_mul(dst[:, dsl], dst[:, dsl], c_sb[:, csl])
                nc.vector.tensor_add(dst[:, dsl], dst[:, dsl], t1[:])

        def proj_chunk_v(hTc, c):
            last = None
            for sti in range(4):
                st = 4 * c + sti
                ps = ps_a.tile([128, CW], f32, tag="mm", name=f"psv{st}")
                for dt in range(NDT):
                    last = nc.tensor.matmul(
                        ps[:], hTc[:, dt * QCH + sti * 128: dt * QCH + (sti + 1) * 128],
                        w_at(wv_sb, dt, 0, CW),
                        start=(dt == 0), stop=(dt == NDT - 1))
                vt = vp.tile([128, CW], bf16, tag="v", name=f"v{st}")
                if use_bias:
                    nc.vector.tensor_add(vt[:], ps[:], bvb_sb[:])
                else:
                    nc.scalar.activation(vt[:], ps[:], IDN)
                v_sb[st] = vt
            return last

        def attention_chunk(qc, qtrc, qoff, W, allow_pop=True,
                            start_anchor=None):
            """Attention for q-window [qc*QCH+qoff, +W); one AllGather piece."""
            qbase = qc * QCH + qoff
            agin = dram.tile([CW, W], bf16, tag=f"agin{qc}_{qoff}",
                             name=f"agin{qc}_{qoff}")
            agout = dram.tile([D, W], bf16, tag=f"agout{qc}_{qoff}",
                              name=f"agout{qc}_{qoff}")
            last_mm = start_anchor
            last_vec = None
            nk = (qbase + W) // 128 if causal else NST
            for h in range(HPC):
                # backfill PE bubbles (rope latency at h==0, exp pacing
                # otherwise) with pending outproj blocks; never during the
                # last chunk -- it delays the tail AllGather
                if allow_pop:
                    for _ in range(len(oblocks) // (HPC - h)):
                        oblocks.pop(0)(last_mm)
                pv = ps_pv.tile([128, W], f32, tag="pv", name=f"pv{h}_{qc}_{qoff}")
                pts = []
                for ki in range(nk):
                    rel = 128 * ki - qbase if causal else -128
                    c0 = max(0, rel)
                    ss = ps_s.tile([128, W], f32, tag="s",
                                   name=f"ss{h}_{qc}_{qoff}_{ki}")
                    nc.tensor.matmul(
                        ss[:, c0:], ktr[h][:, ki * 128:(ki + 1) * 128],
                        qtrc[h][:, qoff + c0:qoff + W], start=True,
                        stop=not (causal and rel >= 0), skip_group_check=True)
                    if causal and rel >= 0:
                        # additive causal mask folded into the accumulation
                        # group: ss[:, c0:c0+128] += tri  (eye^T @ tri = tri)
                        nc.tensor.matmul(ss[:, c0:c0 + 128], eye_sb[:],
                                         tri_sb[:], start=False, stop=True,
                                         skip_group_check=True)
                    if not causal:
                        mt = ptp.tile([128, W], bf16, tag="mt",
                                      name=f"mt{h}_{qc}_{ki}", bufs=2)
                        nc.sync.dma_start(
                            mt[:], mT_d[ki * 128:(ki + 1) * 128,
                                        qbase:qbase + W])
                        nc.vector.tensor_add(ss[:], ss[:], mt[:])
                    pt = ptp.tile([128, W], bf16, tag="pt",
                                  name=f"pt{h}_{qc}_{qoff}_{ki}",
                                  bufs=12 if causal else 8)
                    nc.scalar.activation(pt[:, c0:], ss[:, c0:], EXP)
                    last_mm = nc.tensor.matmul(
                        pv[:, c0:], v_sb[ki][:, h * 128:(h + 1) * 128],
                        pt[:, c0:], start=(ki == 0), stop=(ki == nk - 1),
                        skip_group_check=True)
                    pts.append((pt, c0))
                # in-place partial-width pairwise tree; the last tile is kept
                # out of the eager tree so only ONE add remains after the
                # final exp (short critical path into the rowsum matmul).
                tail = pts[-1]
                pts = pts[:-1]
                while len(pts) > 1:
                    nxt = []
                    for i in range(0, len(pts) - 1, 2):
                        (a, ca), (b, cb) = pts[i], pts[i + 1]
                        nc.vector.tensor_add(a[:, cb:], a[:, cb:], b[:, cb:])
                        nxt.append((a, ca))
                    if len(pts) % 2:
                        nxt.append(pts[-1])
                    pts = nxt
                root = pts[0][0] if pts else tail[0]
                if pts and tail is not None:
                    nc.vector.tensor_add(root[:, tail[1]:], root[:, tail[1]:],
                                         tail[0][:, tail[1]:])
                # partition-reduce+broadcast the rowsums in one bf16 matmul
                sm = ps_s.tile([128, W], f32, tag="s", name=f"sm{h}_{qc}_{qoff}")
                nc.tensor.matmul(sm[:], ones_sb[:], root[:],
                                 start=True, stop=True)
                recb = op.tile([128, W], f32, tag="recb",
                               name=f"recb{h}_{qc}_{qoff}", bufs=2)
                nc.vector.reciprocal_approx_fast(out=recb[:], in_=sm[:])
                ot = op.tile([128, W], bf16, tag="ot", name=f"ot{h}_{qc}_{qoff}",
                             bufs=HPC + 2)
                last_vec = nc.vector.tensor_mul(ot[:], pv[:], recb[:])
                nc.sync.dma_start(agin[h * 128:(h + 1) * 128, :], ot[:])

            nc.gpsimd.collective_compute(
                "AllGather", mybir.AluOpType.bypass,
                replica_groups=REPLICA_GROUPS,
                ins=[agin[:].opt()], outs=[agout[:].opt()])
            return (agout, qoff, W), last_mm, last_vec

        oblocks = []          # pending outproj st4-block emitters (FIFO)

        def push_outproj(qc, piece):
            """Emit the xt reload DMAs for one AllGather piece and queue its
            outproj blocks for interleaving into later attention heads."""
            agout, o, w = piece
            xts = []
            for half in range(2):
                xt = xp.tile([128, 8 * w], bf16, tag="xt",
                             name=f"xt{qc}_{o}_{half}",
                             bufs=3 if causal else 2)
                nc.sync.dma_start(
                    xt[:].rearrange("p (g n) -> p g n", g=8),
                    blk(agout[half * 8 * 128:(half + 1) * 8 * 128, :], 8))
                xts.append(xt)

            def make_block(st4):
                def emit(after_mm):
                    ps = ps_a.tile([128, CW], f32, tag="mm",
                                   name=f"pso{qc}_{o}_{st4}")
                    last = None
                    for dt in range(NDT):
                        mm = last = nc.tensor.matmul(
                            ps[:],
                            xts[dt // 8][:, (dt % 8) * w + st4 * 128:
                                         (dt % 8) * w + (st4 + 1) * 128],
                            w_at(wo_sb, dt, 0, CW),
                            start=(dt == 0), stop=(dt == NDT - 1))
                        if dt == 0 and after_mm is not None:
                            # keep this block behind the attention matmuls
                            # it is meant to backfill (the static scheduler
                            # underestimates AllGather latency and would
                            # hoist it otherwise)
                            add_dep_helper(mm.ins, after_mm.ins, sync=True,
                                           reason="outproj backfill order")
                    row = qc * QCH + o + st4 * 128
                    of = op.tile([128, CW], f32, tag="of",
                                 name=f"of{qc}_{o}_{st4}", bufs=2)
                    if use_bias:
                        nc.vector.tensor_add(of[:], ps[:], bob_sb[:])
                    else:
                        nc.scalar.activation(of[:], ps[:], IDN)
                    nc.sync.dma_start(out_d[row:row + 128, :], of[:])
                    return last
                return emit

            for st4 in range(w // 128):
                oblocks.append(make_block(st4))

        # ---- main pipeline over q-chunks ----
        pieces, last_mms = {}, {}
        qtc_bufs = HPC + 1 if causal else 4 * HPC

        def proj_block(c, after_vec):
            if c + 1 < NQC:
                nsl = slice((c + 1) * QCH, (c + 2) * QCH)
                nxt = hp.tile([128, NDT * QCH], bf16, tag="hT",
                              name=f"hT{c + 1}")
                nc.sync.dma_start(
                    nxt[:].rearrange("p (g n) -> p g n", g=NDT),
                    blk(hT_d[:, nsl], NDT))
                hTs[c + 1] = nxt
            hTc = hTs[c][:]
            # K first: its RoPE chain (vector) overlaps the V+Q matmuls, so
            # the vector queue is clear when attention needs it.
            proj_chunk_qk(wk_sb, bk_sb if use_bias else None, ck_sb, sk_sb,
                          ktr, [slice(c * QCH, (c + 1) * QCH)] * HPC,
                          hTc, c, "k", after_vec)
            qtrc = [qkp.tile([128, QCH], bf16, tag="qtc", name=f"qtc{c}_{m}",
                             bufs=qtc_bufs) for m in range(HPC)]
            proj_chunk_qk(wq_sb, bq_sb if use_bias else None, cq_sb, sq_sb,
                          qtrc, [slice(0, QCH)] * HPC, hTc, c, "q")
            # V last: its matmuls cover the K/Q RoPE chains before attention
            vlast = proj_chunk_v(hTc, c)
            return qtrc, vlast

        HW = QCH // 2
        if causal:
            av = None
            for c in range(NQC):
                qtrc, vlast = proj_block(c, av)
                if c < NQC - 1:
                    pc, lm, av = attention_chunk(c, qtrc, 0, QCH,
                                                 start_anchor=vlast)
                    push_outproj(c, pc)
                else:
                    # split the last chunk so its first AllGather starts a
                    # half-chunk earlier and the tail shrinks
                    pa, _, _ = attention_chunk(c, qtrc, 0, HW, False)
                    push_outproj(c, pa)
                    pb, lm, av = attention_chunk(c, qtrc, HW, HW, False)
                    push_outproj(c, pb)
            prev = lm
            for emit in oblocks:
                prev = emit(prev)
            oblocks.clear()
        else:
            # non-causal: attention(c) needs the FULL K/V, so project
            # everything first, then run the attention/AG/outproj pipeline
            pv_list = [proj_block(c, None) for c in range(NQC)]
            for c in range(NQC):
                qtrc, vlast = pv_list[c]
                if c < NQC - 1:
                    pc, lm, _ = attention_chunk(c, qtrc, 0, QCH,
                                                start_anchor=vlast)
                    push_outproj(c, pc)
                else:
                    pa, _, _ = attention_chunk(c, qtrc, 0, HW, False)
                    push_outproj(c, pa)
                    pb, lm, _ = attention_chunk(c, qtrc, HW, HW, False)
                    push_outproj(c, pb)
            prev = lm
            for emit in oblocks:
                prev = emit(prev)
            oblocks.clear()

    nc.compile()
    return nc


def _get_built(causal: bool, use_bias: bool):
    key = (causal, use_bias)
    if key not in _built:
        _built[key] = _build(causal, use_bias)
    return _built[key]


def _prep_inputs(inputs, causal, use_bias):
    hs = np.asarray(inputs["hidden_states"], np.float32)
    fc = np.asarray(inputs["freqs_cis"], np.float32)
    Wq = np.asarray(inputs["Wq"], np.float32)
    Wk = np.asarray(inputs["Wk"], np.float32)
    Wv = np.asarray(inputs["Wv"], np.float32)
    Wo = np.asarray(inputs["Wo"], np.float32)
    bq = np.asarray(inputs["bq"], np.float32)
    bk = np.asarray(inputs["bk"], np.float32)
    bv = np.asarray(inputs["bv"], np.float32)
    bo = np.asarray(inputs["bo"], np.float32)

    # de-interleave permutation per 128-row head block: [0,2,..,126, 1,3,..,127]
    perm1 = np.concatenate([np.arange(0, DH, 2), np.arange(1, DH, 2)])
    permC = (np.arange(CW) // DH) * DH  # head base offsets
    perm = permC + perm1[np.arange(CW) % DH]

    scale = 1.0 / math.sqrt(DH)
    cos = np.concatenate([fc[:, :, 0].T, fc[:, :, 0].T])  # [128, S], dup halves
    sinh = fc[:, :, 1].T                                  # [64, S]
    sin2 = np.concatenate([-sinh, sinh])                  # [128, S], sign-folded
    cq = np.ascontiguousarray(cos * scale).astype(BF)
    sq = np.ascontiguousarray(sin2 * scale).astype(BF)
    ck = np.ascontiguousarray(cos).astype(BF)
    sk = np.ascontiguousarray(sin2).astype(BF)
    eye = np.eye(128, dtype=np.float32).astype(BF)

    if causal:
        tri = np.where(np.arange(128)[:, None] > np.arange(128)[None, :],
                       np.float32(NEG), np.float32(0.0)).astype(BF)
    else:
        maskT = np.ascontiguousarray(
            np.asarray(inputs["mask"], np.float32)[0, 0].T).astype(BF)

    hTb = [np.ascontiguousarray(hs[b].T).astype(BF) for b in range(B)]

    in_maps = []
    for c in range(NCORES):
        b, hg = divmod(c, GPC)
        sl = slice(CW * hg, CW * (hg + 1))
        wq_s = Wq[sl][perm]
        wk_s = Wk[sl][perm]
        m = {
            "hiddenT": hTb[b],
            "wqT": np.ascontiguousarray(wq_s.T).astype(BF),
            "wkT": np.ascontiguousarray(wk_s.T).astype(BF),
            "wvT": np.ascontiguousarray(Wv[sl].T).astype(BF),
            "woT": np.ascontiguousarray(Wo[sl].T).astype(BF),
            "cq": cq, "sq": sq, "ck": ck, "sk": sk, "eye128": eye,
        }
        if use_bias:
            m["bqp"] = np.ascontiguousarray(
                bq[sl][perm].reshape(HPC, 128).T).astype(np.float32)
            m["bkp"] = np.ascontiguousarray(
                bk[sl][perm].reshape(HPC, 128).T).astype(np.float32)
            m["bv2"] = bv[sl].reshape(1, CW).astype(np.float32)
            m["bo2"] = bo[sl].reshape(1, CW).astype(np.float32)
        if causal:
            m["dmask"] = tri
        else:
            m["maskT"] = maskT
        in_maps.append(m)
    return in_maps


def _is_causal(mask):
    mask = np.asarray(mask, np.float32)
    if mask.shape != (1, 1, S, S):
        return False
    m = mask[0, 0]
    expect = np.triu(np.full((S, S), np.float32(NEG)), k=1)
    return bool(np.array_equal(m, expect))


def run_on_cores(inputs, trace=False):
    """Compile+run; returns BassKernelResults."""
    from concourse.bass_utils import run_bass_kernel_spmd
    causal = _is_causal(inputs["mask"])
    use_bias = any(
        np.any(np.asarray(inputs[k])) for k in ("bq", "bk", "bv", "bo"))
    in_maps = _prep_inputs(inputs, causal, use_bias)
    nc = _get_built(causal, use_bias)
    r = run_bass_kernel_spmd(nc, in_maps, list(range(NCORES)), trace=trace)
    return r


def kernel(**inputs) -> np.ndarray:
    r = run_on_cores(inputs)
    out = np.empty((B, S, D), np.float32)
    for c in range(NCORES):
        b, hg = divmod(c, GPC)
        out[b, :, CW * hg:CW * (hg + 1)] = r.results[c]["out"]
    return out
